# revision 1
# baseline (speedup 1.0000x reference)
"""CapsLayer2D dynamic-routing kernel for 8 Trainium2 NeuronCores.

Full inputs:  inputs [32,14,14,32,8] f32, W [16,32,8,16] f32
Full output:  out [32,14,14,16,16] f32

Sharding: pure data parallel over batch (4 batches / core -> 784 routing
locations per core). W replicated (rearranged host-side into a dense
[256,256] "sum" matrix and a block-diagonal [256,8192] matrix).
"""

import sys

sys.path.insert(0, "/opt/trn_rl_repo")

import numpy as np

import concourse.bass as bass
import concourse.mybir as mybir
from concourse.bacc import Bacc
from concourse.tile import TileContext

F32 = mybir.dt.float32
ADD = mybir.AluOpType.add
MULT = mybir.AluOpType.mult
AX = mybir.AxisListType.X
EXP = mybir.ActivationFunctionType.Exp
SQRT = mybir.ActivationFunctionType.Sqrt

EPS = 1e-7
B, R, C, N, I = 32, 14, 14, 32, 8
K, J = 16, 16
NCORES = 8
BC = B // NCORES            # batches per core
L = BC * R * C              # 784 locations per core
PT = 112                    # locations per partition-tile
NT = L // PT                # 7 tiles
NI = N * I                  # 256
KJ = K * J                  # 256
KN = K * N                  # 512
KNJ = K * N * J             # 8192


def _ap(base, dims):
    """AP over tile `base` ([part, free] contiguous) with free dims
    [(step,count)...] in elements; step 0 = broadcast."""
    return bass.AP(base.tensor, base.offset, [list(base.ap[0])] + [list(d) for d in dims])


def build_bass():
    nc = Bacc()
    x_d = nc.declare_dram_parameter("x", [L, NI], F32, isOutput=False)
    wsum_d = nc.declare_dram_parameter("wsum", [2, 128, KJ], F32, isOutput=False)
    wbd_d = nc.declare_dram_parameter("wbd", [2, 128, KNJ // 2], F32, isOutput=False)
    eye_d = nc.declare_dram_parameter("eye", [128, 128], F32, isOutput=False)
    out_d = nc.declare_dram_parameter("out", [L, KJ], F32, isOutput=True)

    with TileContext(nc) as tc:
        import contextlib
        ctx = contextlib.ExitStack()
        with ctx:
            cpool = ctx.enter_context(tc.tile_pool(name="const", bufs=1))
            wpool = ctx.enter_context(tc.tile_pool(name="work", bufs=2))
            bigpool = ctx.enter_context(tc.tile_pool(name="big", bufs=2))
            tmppool = ctx.enter_context(tc.tile_pool(name="tmp", bufs=1))
            pspool = ctx.enter_context(tc.tile_pool(name="ps", bufs=2, space="PSUM"))
            psmm = ctx.enter_context(tc.tile_pool(name="psmm", bufs=3, space="PSUM"))

            wsum0 = cpool.tile([128, KJ], F32)
            wsum1 = cpool.tile([128, KJ], F32)
            wbd0 = cpool.tile([128, KNJ // 2], F32)
            wbd1 = cpool.tile([128, KNJ // 2], F32)
            eye = cpool.tile([128, 128], F32)
            nc.gpsimd.dma_start(wsum0[:], wsum_d[0])
            nc.gpsimd.dma_start(wsum1[:], wsum_d[1])
            nc.gpsimd.dma_start(wbd0[:], wbd_d[0])
            nc.gpsimd.dma_start(wbd1[:], wbd_d[1])
            nc.gpsimd.dma_start(eye[:], eye_d[:])

            # PE warm-up: absorb the eye/wsum/wbd DMA ticks into PE's vector
            # clock one at a time, so no later LDWEIGHTS needs >1 sync wait
            # (HW limit: one wait slot on LDW).
            ps_w = pspool.tile([128, 512], F32, tag="psT", name="ps_w")
            nc.tensor.transpose(ps_w[:, :128], eye[:], eye[:])
            for wt in (wsum0, wsum1):
                nc.tensor.matmul(ps_w[:, :KJ], wt[:, :128], wt[:],
                                 start=True, stop=True)
            for wt in (wbd0, wbd1):
                nc.tensor.matmul(ps_w[:], wt[:, :128], wt[:, :512],
                                 start=True, stop=True)

            def squash(s_sb, out_sb, tag):
                """out = squash(s) over j; s_sb/out_sb [PT, KJ] f32 (k-major)."""
                tmp_s = wpool.tile([PT, KJ], F32, tag=f"sqt{tag}", name=f"sqt{tag}")
                sq = wpool.tile([PT, K], F32, tag=f"sq{tag}", name=f"sq{tag}")
                den = wpool.tile([PT, K], F32, tag=f"den{tag}", name=f"den{tag}")
                rden = wpool.tile([PT, K], F32, tag=f"rden{tag}", name=f"rden{tag}")
                rt = wpool.tile([PT, K], F32, tag=f"rt{tag}", name=f"rt{tag}")
                rti = wpool.tile([PT, K], F32, tag=f"rti{tag}", name=f"rti{tag}")
                rrt = wpool.tile([PT, K], F32, tag=f"rrt{tag}", name=f"rrt{tag}")
                f = wpool.tile([PT, K], F32, tag=f"f{tag}", name=f"f{tag}")
                nc.vector.tensor_tensor(tmp_s[:], s_sb[:], s_sb[:], MULT)
                nc.vector.tensor_reduce(
                    sq[:], _ap(tmp_s, [[J, K], [1, J]]), AX, ADD)
                nc.scalar.add(den[:], sq[:], 1.0)
                nc.vector.reciprocal(rden[:], den[:])
                nc.vector.tensor_scalar(rti[:], sq[:], EPS, None, ADD)
                nc.scalar.activation(rt[:], rti[:], SQRT)
                nc.vector.reciprocal(rrt[:], rt[:])
                nc.vector.tensor_tensor(f[:], sq[:], rden[:], MULT)
                nc.vector.tensor_tensor(f[:], f[:], rrt[:], MULT)
                nc.vector.tensor_tensor(
                    _ap(out_sb, [[J, K], [1, J]]),
                    _ap(s_sb, [[J, K], [1, J]]),
                    _ap(f, [[1, K], [0, J]]),
                    MULT)

            for t in range(NT):
                x_sb = wpool.tile([PT, NI], F32, tag="x", name="x_sb", bufs=NT)
                nc.gpsimd.dma_start(x_sb[:], x_d[t * PT:(t + 1) * PT, :])

                # transpose x -> xT halves [128, PT]
                xt = []
                for h in range(2):
                    ps_t = pspool.tile([128, PT], F32, tag="psT", name="ps_t")
                    xth = wpool.tile([128, PT], F32, tag=f"xT{h}", name=f"xT{h}")
                    nc.tensor.transpose(
                        ps_t[:], x_sb[:, h * 128:(h + 1) * 128], eye[:PT, :PT])
                    nc.scalar.copy(xth[:], ps_t[:])
                    xt.append(xth)

                # predicted p2 [PT, (k n j)] via block-diag W; chunk ch = n-pair
                p2 = bigpool.tile([PT, KNJ], F32, tag="p2", name="p2")
                for ch in range(16):
                    h = ch // 8
                    wb = (wbd0, wbd1)[h]
                    ps = psmm.tile([PT, 512], F32, tag="mm", name="ps_mm")
                    nc.tensor.matmul(
                        ps[:], xt[h][:], wb[:, (ch % 8) * 512:(ch % 8 + 1) * 512],
                        start=True, stop=True)
                    # psum cols (d,k,j) -> p2 cols k*512 + (2ch+d)*16 + j
                    dst = bass.AP(p2.tensor, p2.offset + 2 * ch * J,
                                  [list(p2.ap[0]), [J, 2], [KN, K], [1, J]])
                    src = _ap(ps, [[KJ, 2], [J, K], [1, J]])
                    eng = nc.scalar if ch % 2 else nc.vector
                    if ch % 2:
                        eng.copy(dst, src)
                    else:
                        eng.tensor_copy(dst, src)

                # iteration 1: c uniform -> s = (x @ wsum)/32
                ps_s = pspool.tile([PT, KJ], F32, tag="s", name="ps_s")
                nc.tensor.matmul(ps_s[:], xt[0][:], wsum0[:], start=True, stop=False)
                nc.tensor.matmul(ps_s[:], xt[1][:], wsum1[:], start=False, stop=True)
                s_sb = wpool.tile([PT, KJ], F32, tag="s_sb", name="s_sb")
                nc.scalar.mul(s_sb[:], ps_s[:], 1.0 / N)
                out_sb = wpool.tile([PT, KJ], F32, tag="out0", name="out_sb")
                squash(s_sb, out_sb, "a")

                b_sb = wpool.tile([PT, KN], F32, tag="b", name="b_sb")
                for it in range(2):
                    # agreement: bn[l,k,n] = sum_j p2[l,k,n,j] * out[l,k,j]
                    tmp = tmppool.tile([PT, KNJ], F32, tag="tmp", name="tmp")
                    nc.vector.tensor_tensor(
                        tmp[:],
                        p2[:],
                        _ap(out_sb, [[J, K], [0, N], [1, J]]),
                        MULT)
                    if it == 0:
                        nc.vector.tensor_reduce(
                            b_sb[:], _ap(tmp, [[J, KN], [1, J]]), AX, ADD)
                    else:
                        bn = wpool.tile([PT, KN], F32, tag="bn", name="bn")
                        nc.vector.tensor_reduce(
                            bn[:], _ap(tmp, [[J, KN], [1, J]]), AX, ADD)
                        nc.vector.tensor_tensor(b_sb[:], b_sb[:], bn[:], ADD)
                    # softmax over n (unnormalized; b bounded, no max-sub)
                    e_sb = wpool.tile([PT, KN], F32, tag="e", name="e_sb")
                    nc.scalar.activation(e_sb[:], b_sb[:], EXP)
                    se = wpool.tile([PT, K], F32, tag="se", name="se")
                    nc.vector.tensor_reduce(
                        se[:], _ap(e_sb, [[N, K], [1, N]]), AX, ADD)
                    r = wpool.tile([PT, K], F32, tag="r", name="r")
                    nc.vector.reciprocal(r[:], se[:])
                    # ws[l,k,j] = sum_n e[l,k,n]*p2[l,k,n,j]; write (k j n) scatter
                    tmp2 = tmppool.tile([PT, KNJ], F32, tag="tmp2", name="tmp2")
                    nc.vector.tensor_tensor(
                        _ap(tmp2, [[KN, K], [1, N], [N, J]]),
                        _ap(p2, [[KN, K], [J, N], [1, J]]),
                        _ap(e_sb, [[N, K], [1, N], [0, J]]),
                        MULT)
                    ws = wpool.tile([PT, KJ], F32, tag="ws", name="ws")
                    nc.vector.tensor_reduce(
                        ws[:], _ap(tmp2, [[N, KJ], [1, N]]), AX, ADD)
                    # s = ws * r (fold softmax normalizer), then squash
                    s2 = wpool.tile([PT, KJ], F32, tag="s2", name="s2")
                    nc.vector.tensor_tensor(
                        _ap(s2, [[J, K], [1, J]]),
                        _ap(ws, [[J, K], [1, J]]),
                        _ap(r, [[1, K], [0, J]]),
                        MULT)
                    out_sb = wpool.tile([PT, KJ], F32, tag=f"out{it + 1}",
                                        name="out_it")
                    squash(s2, out_sb, f"i{it}")

                nc.gpsimd.dma_start(out_d[t * PT:(t + 1) * PT, :], out_sb[:])
    nc.compile()
    return nc


def host_prep(inputs, W):
    x = np.ascontiguousarray(inputs, np.float32).reshape(NCORES, L, NI)
    wsum = np.ascontiguousarray(
        W.transpose(1, 2, 0, 3).reshape(NI, KJ), np.float32).reshape(2, 128, KJ)
    wbd_full = np.zeros((NI, KNJ), np.float32)
    for n in range(N):
        wbd_full[n * I:(n + 1) * I, n * KJ:(n + 1) * KJ] = (
            W[:, n].transpose(1, 0, 2).reshape(I, KJ))
    wbd = np.stack([wbd_full[0:128, 0:KNJ // 2],
                    wbd_full[128:256, KNJ // 2:]])
    eye = np.eye(128, dtype=np.float32)
    return x, wsum, wbd, eye


_CACHED = {}


def kernel(inputs, W):
    from concourse.bass_utils import run_bass_kernel_spmd

    x, wsum, wbd, eye = host_prep(inputs, W)
    if "nc" not in _CACHED:
        _CACHED["nc"] = build_bass()
    nc = _CACHED["nc"]
    in_maps = [{"x": np.ascontiguousarray(x[c]), "wsum": wsum, "wbd": wbd,
                "eye": eye} for c in range(NCORES)]
    res = run_bass_kernel_spmd(nc, in_maps, core_ids=list(range(NCORES)))
    out = np.stack([res.results[c]["out"] for c in range(NCORES)])
    return out.reshape(B, R, C, K, J)



# revision 8
# speedup vs baseline: 1.4727x; 1.4727x over previous
"""CapsLayer2D dynamic-routing kernel for 8 Trainium2 NeuronCores.

Full inputs:  inputs [32,14,14,32,8] f32, W [16,32,8,16] f32
Full output:  out [32,14,14,16,16] f32

Sharding: pure data parallel over batch (4 batches / core -> 784 routing
locations per core). W replicated (rearranged host-side into a dense
[256,256] "sum" matrix and a block-diagonal [256,8192] matrix, fp16).

v2: fp16 datapath. Big elementwise ops run in DVE 2x_1p mode (2-byte
dtypes, packed innermost); TENSOR_REDUCE (no fast mode) replaced by
TT-add trees; the ws multiply (broadcast innermost, can't pack) goes to
gpsimd; exp/sqrt/copies on the scalar engine; DMA issue on sync.
"""

import sys

sys.path.insert(0, "/opt/trn_rl_repo")

import numpy as np

import concourse.bass as bass
import concourse.mybir as mybir
from concourse.bacc import Bacc
from concourse.tile import TileContext

F32 = mybir.dt.float32
F16 = mybir.dt.float16
ADD = mybir.AluOpType.add
MULT = mybir.AluOpType.mult
AX = mybir.AxisListType.X
EXP = mybir.ActivationFunctionType.Exp
SQRT = mybir.ActivationFunctionType.Sqrt
SQUARE = mybir.ActivationFunctionType.Square

EPS = 1e-7
B, R, C, N, I = 32, 14, 14, 32, 8
K, J = 16, 16
NCORES = 8
BC = B // NCORES            # batches per core
L = BC * R * C              # 784 locations per core
PT = 112                    # locations per partition-tile
NT = L // PT                # 7 tiles
NI = N * I                  # 256
KJ = K * J                  # 256
NJ = N * J                  # 512 (= k-stride in p2)
KN = K * N                  # 512
KNJ = K * N * J             # 8192


def _ap(base, dims):
    """AP over tile `base` ([part, free] contiguous) with free dims
    [(step,count)...] in elements; step 0 = broadcast."""
    return bass.AP(base.tensor, base.offset, [list(base.ap[0])] + [list(d) for d in dims])


def _apo(base, off, dims):
    """Same, with an element offset into the free dim."""
    return bass.AP(base.tensor, base.offset + off,
                   [list(base.ap[0])] + [list(d) for d in dims])


def build_bass():
    nc = Bacc()
    x_d = nc.declare_dram_parameter("x", [L, NI], F16, isOutput=False)
    wsum_d = nc.declare_dram_parameter("wsum", [2, 128, KJ], F16, isOutput=False)
    wbd_d = nc.declare_dram_parameter("wbd", [2, 128, KNJ // 2], F16, isOutput=False)
    eye_d = nc.declare_dram_parameter("eye", [128, 128], F16, isOutput=False)
    out_d = nc.declare_dram_parameter("out", [L, KJ], F32, isOutput=True)

    with TileContext(nc) as tc:
        import contextlib
        ctx = contextlib.ExitStack()
        with ctx:
            cpool = ctx.enter_context(tc.tile_pool(name="const", bufs=1))
            wpool = ctx.enter_context(tc.tile_pool(name="work", bufs=2))
            bigpool = ctx.enter_context(tc.tile_pool(name="big", bufs=2))
            tpool = ctx.enter_context(tc.tile_pool(name="tree", bufs=1))
            pspool = ctx.enter_context(tc.tile_pool(name="ps", bufs=2, space="PSUM"))
            pswarm = ctx.enter_context(tc.tile_pool(name="psw", bufs=1, space="PSUM"))
            psmm = ctx.enter_context(tc.tile_pool(name="psmm", bufs=3, space="PSUM"))

            wsum0 = cpool.tile([128, KJ], F16)
            wsum1 = cpool.tile([128, KJ], F16)
            wbd0 = cpool.tile([128, KNJ // 2], F16)
            wbd1 = cpool.tile([128, KNJ // 2], F16)
            eye = cpool.tile([128, 128], F16)
            nc.sync.dma_start(wsum0[:], wsum_d[0])
            nc.sync.dma_start(wsum1[:], wsum_d[1])
            nc.sync.dma_start(wbd0[:], wbd_d[0])
            nc.sync.dma_start(wbd1[:], wbd_d[1])
            nc.sync.dma_start(eye[:], eye_d[:])

            # PE warm-up: absorb the eye/wsum/wbd DMA ticks into PE's vector
            # clock one at a time, so no later LDWEIGHTS needs >1 sync wait
            # (HW limit: one wait slot on LDW).
            ps_w = pswarm.tile([128, 512], F32, tag="psW", name="ps_w")
            ps_wt = pspool.tile([128, 128], F16, tag="psT", name="ps_wt")
            nc.tensor.transpose(ps_wt[:], eye[:], eye[:])
            for wt in (wsum0, wsum1):
                nc.tensor.matmul(ps_w[:, :KJ], wt[:, :128], wt[:],
                                 start=True, stop=True)
            for wt in (wbd0, wbd1):
                nc.tensor.matmul(ps_w[:], wt[:, :128], wt[:, :512],
                                 start=True, stop=True)

            def squash(s_sb, out_sb, tag):
                """out = squash(s) over j; s_sb/out_sb [PT, KJ] f32 (k-major).
                f = sq/((1+sq)*sqrt(sq+eps)); out = s*f."""
                ssq = wpool.tile([PT, KJ], F32, tag=f"ssq{tag}", name=f"ssq{tag}")
                sq = wpool.tile([PT, K], F32, tag=f"sq{tag}", name=f"sq{tag}")
                rti = wpool.tile([PT, K], F32, tag=f"rti{tag}", name=f"rti{tag}")
                rt = wpool.tile([PT, K], F32, tag=f"rt{tag}", name=f"rt{tag}")
                den = wpool.tile([PT, K], F32, tag=f"den{tag}", name=f"den{tag}")
                rden = wpool.tile([PT, K], F32, tag=f"rd{tag}", name=f"rd{tag}")
                f = wpool.tile([PT, K], F32, tag=f"f{tag}", name=f"f{tag}")
                nc.scalar.activation(ssq[:], s_sb[:], SQUARE)
                nc.vector.tensor_reduce(
                    sq[:], _ap(ssq, [[J, K], [1, J]]), AX, ADD)
                nc.vector.tensor_scalar(rti[:], sq[:], EPS, None, ADD)
                nc.scalar.activation(rt[:], rti[:], SQRT)
                nc.scalar.add(den[:], sq[:], 1.0)
                nc.vector.tensor_tensor(den[:], den[:], rt[:], MULT)
                nc.vector.reciprocal(rden[:], den[:])
                nc.vector.tensor_tensor(f[:], sq[:], rden[:], MULT)
                nc.vector.tensor_tensor(
                    _ap(out_sb, [[J, K], [1, J]]),
                    _ap(s_sb, [[J, K], [1, J]]),
                    _ap(f, [[1, K], [0, J]]),
                    MULT)

            # psum->sbuf p2 copy engine pattern: scalar-heavy (gpsimd cannot
            # read PSUM)
            cp_eng = [nc.scalar, nc.vector, nc.scalar] * 6

            for t in range(NT):
                x_sb = wpool.tile([PT, NI], F16, tag="x", name="x_sb", bufs=NT)
                nc.sync.dma_start(x_sb[:], x_d[t * PT:(t + 1) * PT, :])

                # transpose x -> xT halves [128, PT] fp16
                xt = []
                for h in range(2):
                    ps_t = pspool.tile([128, PT], F16, tag="psT", name="ps_t")
                    xth = wpool.tile([128, PT], F16, tag=f"xT{h}", name=f"xT{h}")
                    nc.tensor.transpose(
                        ps_t[:], x_sb[:, h * 128:(h + 1) * 128], eye[:PT, :PT])
                    nc.scalar.copy(xth[:], ps_t[:])
                    xt.append(xth)

                # predicted p2 [PT, (k n j)] fp16 via block-diag W; ch = n-pair
                p2 = bigpool.tile([PT, KNJ], F16, tag="p2", name="p2")
                for ch in range(16):
                    h = ch // 8
                    wb = (wbd0, wbd1)[h]
                    ps = psmm.tile([PT, 512], F32, tag="mm", name="ps_mm")
                    nc.tensor.matmul(
                        ps[:], xt[h][:], wb[:, (ch % 8) * 512:(ch % 8 + 1) * 512],
                        start=True, stop=True)
                    # psum cols (d,k,j) -> p2 cols k*512 + (2ch+d)*16 + j
                    dst = _apo(p2, 2 * ch * J, [[J, 2], [NJ, K], [1, J]])
                    src = _ap(ps, [[KJ, 2], [J, K], [1, J]])
                    eng = cp_eng[ch]
                    if eng is nc.scalar:
                        eng.copy(dst, src)
                    else:
                        eng.tensor_copy(dst, src)

                # iteration 1: c uniform -> s = (x @ wsum)/32
                ps_s = pspool.tile([PT, KJ], F32, tag="s", name="ps_s")
                nc.tensor.matmul(ps_s[:], xt[0][:], wsum0[:], start=True, stop=False)
                nc.tensor.matmul(ps_s[:], xt[1][:], wsum1[:], start=False, stop=True)
                s_sb = wpool.tile([PT, KJ], F32, tag="s_sb", name="s_sb")
                nc.scalar.mul(s_sb[:], ps_s[:], 1.0 / N)
                out_sb = wpool.tile([PT, KJ], F32, tag="out0", name="out_sb")
                squash(s_sb, out_sb, "a")
                out_h = wpool.tile([PT, KJ], F16, tag="oh0", name="out_h")
                nc.scalar.copy(out_h[:], out_sb[:])

                b_sb = wpool.tile([PT, KN], F32, tag="b", name="b_sb")
                for it in range(2):
                    # ---- agreement bn[l,k,n] = sum_j p2*out, as fp16 tree ----
                    tmp = bigpool.tile([PT, KNJ], F16, tag="tmp", name="tmp")
                    nc.vector.tensor_tensor(
                        tmp[:], p2[:],
                        _ap(out_h, [[J, K], [0, N], [1, J]]),
                        MULT)
                    t1 = tpool.tile([PT, KNJ // 2], F16, tag="t1", name="t1")
                    nc.vector.tensor_tensor(
                        _ap(t1, [[8, KN], [1, 8]]),
                        _ap(tmp, [[J, KN], [1, 8]]),
                        _apo(tmp, 8, [[J, KN], [1, 8]]),
                        ADD)
                    t2 = tpool.tile([PT, KNJ // 4], F16, tag="t2", name="t2")
                    nc.vector.tensor_tensor(
                        _ap(t2, [[4, KN], [1, 4]]),
                        _ap(t1, [[8, KN], [1, 4]]),
                        _apo(t1, 4, [[8, KN], [1, 4]]),
                        ADD)
                    t3 = tpool.tile([PT, KNJ // 8], F16, tag="t3", name="t3")
                    nc.vector.tensor_tensor(
                        _ap(t3, [[2, KN], [1, 2]]),
                        _ap(t2, [[4, KN], [1, 2]]),
                        _apo(t2, 2, [[4, KN], [1, 2]]),
                        ADD)
                    if it == 0:
                        nc.vector.tensor_tensor(
                            b_sb[:],
                            _ap(t3, [[2, KN]]),
                            _apo(t3, 1, [[2, KN]]),
                            ADD)
                    else:
                        bn = wpool.tile([PT, KN], F32, tag="bn", name="bn")
                        nc.vector.tensor_tensor(
                            bn[:],
                            _ap(t3, [[2, KN]]),
                            _apo(t3, 1, [[2, KN]]),
                            ADD)
                        nc.vector.tensor_tensor(b_sb[:], b_sb[:], bn[:], ADD)

                    # ---- softmax over n (b bounded; no max-sub), c = e*r ----
                    e_sb = wpool.tile([PT, KN], F32, tag="e", name="e_sb")
                    nc.scalar.activation(e_sb[:], b_sb[:], EXP)
                    se = wpool.tile([PT, K], F32, tag="se", name="se")
                    nc.vector.tensor_reduce(
                        se[:], _ap(e_sb, [[N, K], [1, N]]), AX, ADD)
                    r = wpool.tile([PT, K], F32, tag="r", name="r")
                    nc.vector.reciprocal(r[:], se[:])
                    c_h = wpool.tile([PT, KN], F16, tag="c", name="c_h")
                    nc.vector.tensor_tensor(
                        c_h[:], e_sb[:],
                        _ap(r, [[1, K], [0, N]]),
                        MULT)

                    # ---- s[l,k,j] = sum_n c*p2: gpsimd mult + fp16 tree ----
                    tmp2 = bigpool.tile([PT, KNJ], F16, tag="tmp2", name="tmp2")
                    nc.gpsimd.tensor_tensor(
                        tmp2[:], p2[:],
                        _ap(c_h, [[N, K], [1, N], [0, J]]),
                        MULT)
                    u1 = tpool.tile([PT, KNJ // 2], F16, tag="u1", name="u1")
                    nc.vector.tensor_tensor(
                        _ap(u1, [[NJ // 2, K], [J, 16], [1, J]]),
                        _ap(tmp2, [[NJ, K], [J, 16], [1, J]]),
                        _apo(tmp2, 16 * J, [[NJ, K], [J, 16], [1, J]]),
                        ADD)
                    u2 = tpool.tile([PT, KNJ // 4], F16, tag="u2", name="u2")
                    nc.vector.tensor_tensor(
                        _ap(u2, [[NJ // 4, K], [J, 8], [1, J]]),
                        _ap(u1, [[NJ // 2, K], [J, 8], [1, J]]),
                        _apo(u1, 8 * J, [[NJ // 2, K], [J, 8], [1, J]]),
                        ADD)
                    u3 = tpool.tile([PT, KNJ // 8], F16, tag="u3", name="u3")
                    nc.vector.tensor_tensor(
                        _ap(u3, [[NJ // 8, K], [J, 4], [1, J]]),
                        _ap(u2, [[NJ // 4, K], [J, 4], [1, J]]),
                        _apo(u2, 4 * J, [[NJ // 4, K], [J, 4], [1, J]]),
                        ADD)
                    u4 = tpool.tile([PT, KNJ // 16], F16, tag="u4", name="u4")
                    nc.vector.tensor_tensor(
                        _ap(u4, [[NJ // 16, K], [J, 2], [1, J]]),
                        _ap(u3, [[NJ // 8, K], [J, 2], [1, J]]),
                        _apo(u3, 2 * J, [[NJ // 8, K], [J, 2], [1, J]]),
                        ADD)
                    s2 = wpool.tile([PT, KJ], F32, tag="s2", name="s2")
                    nc.vector.tensor_tensor(
                        _ap(s2, [[J, K], [1, J]]),
                        _ap(u4, [[2 * J, K], [1, J]]),
                        _apo(u4, J, [[2 * J, K], [1, J]]),
                        ADD)
                    out_sb = wpool.tile([PT, KJ], F32, tag=f"out{it + 1}",
                                        name="out_it")
                    squash(s2, out_sb, f"i{it}")
                    if it == 0:
                        out_h = wpool.tile([PT, KJ], F16, tag="oh1", name="out_h1")
                        nc.scalar.copy(out_h[:], out_sb[:])

                nc.sync.dma_start(out_d[t * PT:(t + 1) * PT, :], out_sb[:])
    nc.compile()
    return nc


def host_prep(inputs, W):
    x = np.ascontiguousarray(inputs, np.float32).reshape(NCORES, L, NI)
    x = x.astype(np.float16)
    wsum = np.ascontiguousarray(
        W.transpose(1, 2, 0, 3).reshape(NI, KJ), np.float32)
    wsum = wsum.astype(np.float16).reshape(2, 128, KJ)
    wbd_full = np.zeros((NI, KNJ), np.float16)
    for n in range(N):
        wbd_full[n * I:(n + 1) * I, n * KJ:(n + 1) * KJ] = (
            W[:, n].transpose(1, 0, 2).reshape(I, KJ).astype(np.float16))
    wbd = np.stack([wbd_full[0:128, 0:KNJ // 2],
                    wbd_full[128:256, KNJ // 2:]])
    eye = np.eye(128, dtype=np.float16)
    return x, wsum, wbd, eye


_CACHED = {}


def kernel(inputs, W):
    from concourse.bass_utils import run_bass_kernel_spmd

    x, wsum, wbd, eye = host_prep(inputs, W)
    if "nc" not in _CACHED:
        _CACHED["nc"] = build_bass()
    nc = _CACHED["nc"]
    in_maps = [{"x": np.ascontiguousarray(x[c]), "wsum": wsum, "wbd": wbd,
                "eye": eye} for c in range(NCORES)]
    res = run_bass_kernel_spmd(nc, in_maps, core_ids=list(range(NCORES)))
    out = np.stack([res.results[c]["out"] for c in range(NCORES)])
    return out.reshape(B, R, C, K, J)


# revision 12
# speedup vs baseline: 1.8375x; 1.2478x over previous
"""CapsLayer2D dynamic-routing kernel for 8 Trainium2 NeuronCores.

Full inputs:  inputs [32,14,14,32,8] f32, W [16,32,8,16] f32
Full output:  out [32,14,14,16,16] f32

Sharding: pure data parallel over batch (4 batches / core -> 784 routing
locations per core). W replicated (rearranged host-side into a dense
[256,256] "sum" matrix and a block-diagonal [256,8192] matrix, fp16).

v3: fp16 datapath; all big elementwise ops on DVE in 2x_1p mode (2-byte
dtypes, packed innermost); TENSOR_REDUCE replaced by TT-add trees; the
ws multiply uses a scalar-engine broadcast expansion (c over j) feeding
a packed DVE multiply, ping-ponged in k-halves; tile t+1's PE/copy
stage is emitted between tile t's routing iterations for overlap.
GpSimd is avoided for big ops (shared SBUF ports starve the DVE).
"""

import sys

sys.path.insert(0, "/opt/trn_rl_repo")

import numpy as np

import concourse.bass as bass
import concourse.mybir as mybir
from concourse.bacc import Bacc
from concourse.tile import TileContext

F32 = mybir.dt.float32
F16 = mybir.dt.float16
ADD = mybir.AluOpType.add
MULT = mybir.AluOpType.mult
AX = mybir.AxisListType.X
EXP = mybir.ActivationFunctionType.Exp
SQRT = mybir.ActivationFunctionType.Sqrt
SQUARE = mybir.ActivationFunctionType.Square

EPS = 1e-7
B, R, C, N, I = 32, 14, 14, 32, 8
K, J = 16, 16
NCORES = 8
BC = B // NCORES            # batches per core
L = BC * R * C              # 784 locations per core
PT = 112                    # locations per partition-tile
NT = L // PT                # 7 tiles
NI = N * I                  # 256
KJ = K * J                  # 256
NJ = N * J                  # 512 (= k-stride in p2)
KN = K * N                  # 512
KNJ = K * N * J             # 8192


def _ap(base, dims):
    """AP over tile `base` ([part, free] contiguous) with free dims
    [(step,count)...] in elements; step 0 = broadcast."""
    return bass.AP(base.tensor, base.offset, [list(base.ap[0])] + [list(d) for d in dims])


def _apo(base, off, dims):
    """Same, with an element offset into the free dim."""
    return bass.AP(base.tensor, base.offset + off,
                   [list(base.ap[0])] + [list(d) for d in dims])


def build_bass():
    nc = Bacc()
    x_d = nc.declare_dram_parameter("x", [L, NI], F16, isOutput=False)
    wsum_d = nc.declare_dram_parameter("wsum", [2, 128, KJ], F16, isOutput=False)
    wbd_d = nc.declare_dram_parameter("wbd", [2, 128, KNJ // 2], F16, isOutput=False)
    eye_d = nc.declare_dram_parameter("eye", [128, 128], F16, isOutput=False)
    out_d = nc.declare_dram_parameter("out", [L, KJ], F32, isOutput=True)

    with TileContext(nc) as tc:
        import contextlib
        ctx = contextlib.ExitStack()
        with ctx:
            cpool = ctx.enter_context(tc.tile_pool(name="const", bufs=1))
            wpool = ctx.enter_context(tc.tile_pool(name="work", bufs=2))
            bigpool = ctx.enter_context(tc.tile_pool(name="big", bufs=2))
            tpool = ctx.enter_context(tc.tile_pool(name="tree", bufs=1))
            pspool = ctx.enter_context(tc.tile_pool(name="ps", bufs=2, space="PSUM"))
            pswarm = ctx.enter_context(tc.tile_pool(name="psw", bufs=1, space="PSUM"))
            psmm = ctx.enter_context(tc.tile_pool(name="psmm", bufs=3, space="PSUM"))

            wsum0 = cpool.tile([128, KJ], F16)
            wsum1 = cpool.tile([128, KJ], F16)
            wbd0 = cpool.tile([128, KNJ // 2], F16)
            wbd1 = cpool.tile([128, KNJ // 2], F16)
            eye = cpool.tile([128, 128], F16)
            nc.sync.dma_start(wsum0[:], wsum_d[0])
            nc.sync.dma_start(wsum1[:], wsum_d[1])
            nc.sync.dma_start(wbd0[:], wbd_d[0])
            nc.sync.dma_start(wbd1[:], wbd_d[1])
            nc.sync.dma_start(eye[:], eye_d[:])

            # PE warm-up: absorb the eye/wsum/wbd DMA ticks into PE's vector
            # clock one at a time, so no later LDWEIGHTS needs >1 sync wait
            # (HW limit: one wait slot on LDW).
            ps_w = pswarm.tile([128, 512], F32, tag="psW", name="ps_w")
            ps_wt = pspool.tile([128, 128], F16, tag="psT", name="ps_wt")
            nc.tensor.transpose(ps_wt[:], eye[:], eye[:])
            for wt in (wsum0, wsum1):
                nc.tensor.matmul(ps_w[:, :KJ], wt[:, :128], wt[:],
                                 start=True, stop=True)
            for wt in (wbd0, wbd1):
                nc.tensor.matmul(ps_w[:], wt[:, :128], wt[:, :512],
                                 start=True, stop=True)

            def squash(s_sb, out_sb, tag):
                """out = squash(s) over j; s_sb/out_sb [PT, KJ] f32 (k-major).
                f = sq/((1+sq)*sqrt(sq+eps)); out = s*f."""
                ssq = wpool.tile([PT, KJ], F32, tag=f"ssq{tag}", name=f"ssq{tag}")
                sq = wpool.tile([PT, K], F32, tag=f"sq{tag}", name=f"sq{tag}")
                rti = wpool.tile([PT, K], F32, tag=f"rti{tag}", name=f"rti{tag}")
                rt = wpool.tile([PT, K], F32, tag=f"rt{tag}", name=f"rt{tag}")
                den = wpool.tile([PT, K], F32, tag=f"den{tag}", name=f"den{tag}")
                rden = wpool.tile([PT, K], F32, tag=f"rd{tag}", name=f"rd{tag}")
                f = wpool.tile([PT, K], F32, tag=f"f{tag}", name=f"f{tag}")
                nc.scalar.activation(ssq[:], s_sb[:], SQUARE)
                nc.vector.tensor_reduce(
                    sq[:], _ap(ssq, [[J, K], [1, J]]), AX, ADD)
                nc.vector.tensor_scalar(rti[:], sq[:], EPS, None, ADD)
                nc.scalar.activation(rt[:], rti[:], SQRT)
                nc.scalar.add(den[:], sq[:], 1.0)
                nc.vector.tensor_tensor(den[:], den[:], rt[:], MULT)
                nc.vector.reciprocal(rden[:], den[:])
                nc.vector.tensor_tensor(f[:], sq[:], rden[:], MULT)
                nc.vector.tensor_tensor(
                    _ap(out_sb, [[J, K], [1, J]]),
                    _ap(s_sb, [[J, K], [1, J]]),
                    _ap(f, [[1, K], [0, J]]),
                    MULT)

            # psum->sbuf p2 copy engines, per chunk (gpsimd cannot read PSUM)
            cp_eng = [nc.scalar, nc.vector, nc.scalar] * 6

            def stage_a1(t):
                """x DMA, transposes, p2 chunks 0-7. Returns state dict."""
                st = {"t": t}
                x_sb = wpool.tile([PT, NI], F16, tag="x", name="x_sb", bufs=3)
                nc.sync.dma_start(x_sb[:], x_d[t * PT:(t + 1) * PT, :])
                xt = []
                for h in range(2):
                    ps_t = pspool.tile([128, PT], F16, tag="psT", name="ps_t")
                    xth = wpool.tile([128, PT], F16, tag=f"xT{h}", name=f"xT{h}")
                    nc.tensor.transpose(
                        ps_t[:], x_sb[:, h * 128:(h + 1) * 128], eye[:PT, :PT])
                    nc.scalar.copy(xth[:], ps_t[:])
                    xt.append(xth)
                st["xt"] = xt
                p2 = bigpool.tile([PT, KNJ], F16, tag="p2", name="p2")
                st["p2"] = p2
                for ch in range(8):
                    _p2_chunk(st, ch)
                return st

            def _p2_chunk(st, ch):
                h = ch // 8
                wb = (wbd0, wbd1)[h]
                ps = psmm.tile([PT, 512], F32, tag="mm", name="ps_mm")
                nc.tensor.matmul(
                    ps[:], st["xt"][h][:],
                    wb[:, (ch % 8) * 512:(ch % 8 + 1) * 512],
                    start=True, stop=True)
                # psum cols (d,k,j) -> p2 cols k*512 + (2ch+d)*16 + j
                dst = _apo(st["p2"], 2 * ch * J, [[J, 2], [NJ, K], [1, J]])
                src = _ap(ps, [[KJ, 2], [J, K], [1, J]])
                eng = cp_eng[ch]
                if eng is nc.scalar:
                    eng.copy(dst, src)
                else:
                    eng.tensor_copy(dst, src)

            def stage_a2(st):
                """p2 chunks 8-15, then iteration 1 (uniform c)."""
                for ch in range(8, 16):
                    _p2_chunk(st, ch)
                xt = st["xt"]
                ps_s = pspool.tile([PT, KJ], F32, tag="s", name="ps_s")
                nc.tensor.matmul(ps_s[:], xt[0][:], wsum0[:], start=True, stop=False)
                nc.tensor.matmul(ps_s[:], xt[1][:], wsum1[:], start=False, stop=True)
                s_sb = wpool.tile([PT, KJ], F32, tag="s_sb", name="s_sb")
                nc.scalar.mul(s_sb[:], ps_s[:], 1.0 / N)
                out_sb = wpool.tile([PT, KJ], F32, tag="out0", name="out_sb")
                squash(s_sb, out_sb, "a")
                out_h = wpool.tile([PT, KJ], F16, tag="oh0", name="out_h")
                nc.scalar.copy(out_h[:], out_sb[:])
                st["out"] = out_sb
                st["out_h"] = out_h

            def routing_iter(st, it):
                p2, out_h = st["p2"], st["out_h"]
                # ---- agreement bn[l,k,n] = sum_j p2*out: fp16 tree ----
                tmp = bigpool.tile([PT, KNJ], F16, tag="tmp", name="tmp")
                nc.vector.tensor_tensor(
                    tmp[:], p2[:],
                    _ap(out_h, [[J, K], [0, N], [1, J]]),
                    MULT)
                t1 = tpool.tile([PT, KNJ // 2], F16, tag="t1", name="t1")
                nc.vector.tensor_tensor(
                    _ap(t1, [[8, KN], [1, 8]]),
                    _ap(tmp, [[J, KN], [1, 8]]),
                    _apo(tmp, 8, [[J, KN], [1, 8]]),
                    ADD)
                t2 = tpool.tile([PT, KNJ // 4], F16, tag="t2", name="t2")
                nc.vector.tensor_tensor(
                    _ap(t2, [[4, KN], [1, 4]]),
                    _ap(t1, [[8, KN], [1, 4]]),
                    _apo(t1, 4, [[8, KN], [1, 4]]),
                    ADD)
                t3 = tpool.tile([PT, KNJ // 8], F16, tag="t3", name="t3")
                nc.vector.tensor_tensor(
                    _ap(t3, [[2, KN], [1, 2]]),
                    _ap(t2, [[4, KN], [1, 2]]),
                    _apo(t2, 2, [[4, KN], [1, 2]]),
                    ADD)
                t4 = tpool.tile([PT, KNJ // 16], F16, tag="t4", name="t4")
                nc.vector.tensor_tensor(
                    _ap(t4, [[1, KN]]),
                    _ap(t3, [[2, KN]]),
                    _apo(t3, 1, [[2, KN]]),
                    ADD)
                if it == 0:
                    b_sb = wpool.tile([PT, KN], F32, tag="b", name="b_sb")
                    nc.vector.tensor_copy(b_sb[:], t4[:])
                    st["b"] = b_sb
                else:
                    b_sb = st["b"]
                    nc.vector.tensor_tensor(b_sb[:], b_sb[:], t4[:], ADD)

                # ---- softmax over n (b bounded; no max-sub), c = e*r ----
                e_sb = wpool.tile([PT, KN], F32, tag="e", name="e_sb")
                nc.scalar.activation(e_sb[:], b_sb[:], EXP)
                se = wpool.tile([PT, K], F32, tag="se", name="se")
                nc.vector.tensor_reduce(
                    se[:], _ap(e_sb, [[N, K], [1, N]]), AX, ADD)
                r = wpool.tile([PT, K], F32, tag="r", name="r")
                nc.vector.reciprocal(r[:], se[:])
                c_h = wpool.tile([PT, KN], F16, tag="c", name="c_h")
                nc.vector.tensor_tensor(
                    c_h[:], e_sb[:],
                    _ap(r, [[1, K], [0, N]]),
                    MULT)

                # ---- s[l,k,j] = sum_n c*p2: expand c over j on the scalar
                # engine (k-halves, ping-ponged with the DVE multiply), then
                # packed fp16 multiply + tree over n on DVE ----
                u1 = tpool.tile([PT, KNJ // 2], F16, tag="u1", name="u1")
                for hf in range(2):
                    cj = tpool.tile([PT, KNJ // 2], F16, tag=f"cj{hf}",
                                    name=f"cj{hf}")
                    nc.scalar.copy(
                        _ap(cj, [[NJ, 8], [J, N], [1, J]]),
                        _apo(c_h, hf * (KN // 2), [[N, 8], [1, N], [0, J]]))
                    t2h = tpool.tile([PT, KNJ // 2], F16, tag=f"m2{hf}",
                                     name=f"m2{hf}")
                    nc.vector.tensor_tensor(
                        t2h[:], _apo(p2, hf * (KNJ // 2), [[1, KNJ // 2]]),
                        cj[:], MULT)
                    # n 32 -> 16 within this half; u1 k-stride 256
                    nc.vector.tensor_tensor(
                        _apo(u1, hf * (KNJ // 4), [[NJ // 2, 8], [J, 16], [1, J]]),
                        _ap(t2h, [[NJ, 8], [J, 16], [1, J]]),
                        _apo(t2h, 16 * J, [[NJ, 8], [J, 16], [1, J]]),
                        ADD)
                u2 = tpool.tile([PT, KNJ // 4], F16, tag="u2", name="u2")
                nc.vector.tensor_tensor(
                    _ap(u2, [[NJ // 4, K], [J, 8], [1, J]]),
                    _ap(u1, [[NJ // 2, K], [J, 8], [1, J]]),
                    _apo(u1, 8 * J, [[NJ // 2, K], [J, 8], [1, J]]),
                    ADD)
                u3 = tpool.tile([PT, KNJ // 8], F16, tag="u3", name="u3")
                nc.vector.tensor_tensor(
                    _ap(u3, [[NJ // 8, K], [J, 4], [1, J]]),
                    _ap(u2, [[NJ // 4, K], [J, 4], [1, J]]),
                    _apo(u2, 4 * J, [[NJ // 4, K], [J, 4], [1, J]]),
                    ADD)
                u4 = tpool.tile([PT, KNJ // 16], F16, tag="u4", name="u4")
                nc.vector.tensor_tensor(
                    _ap(u4, [[NJ // 16, K], [J, 2], [1, J]]),
                    _ap(u3, [[NJ // 8, K], [J, 2], [1, J]]),
                    _apo(u3, 2 * J, [[NJ // 8, K], [J, 2], [1, J]]),
                    ADD)
                s2 = wpool.tile([PT, KJ], F32, tag="s2", name="s2")
                nc.vector.tensor_tensor(
                    _ap(s2, [[J, K], [1, J]]),
                    _ap(u4, [[2 * J, K], [1, J]]),
                    _apo(u4, J, [[2 * J, K], [1, J]]),
                    ADD)
                out_sb = wpool.tile([PT, KJ], F32, tag=f"out{it + 1}",
                                    name="out_it")
                squash(s2, out_sb, f"i{it}")
                st["out"] = out_sb
                if it == 0:
                    out_h = wpool.tile([PT, KJ], F16, tag="oh1", name="out_h1")
                    nc.scalar.copy(out_h[:], out_sb[:])
                    st["out_h"] = out_h

            # software-pipelined schedule: tile t+1's PE/copy stages are
            # emitted between tile t's routing iterations.
            st = stage_a1(0)
            stage_a2(st)
            nxt = stage_a1(1) if NT > 1 else None
            for t in range(NT):
                routing_iter(st, 0)
                if nxt is not None:
                    stage_a2(nxt)
                routing_iter(st, 1)
                nc.sync.dma_start(out_d[t * PT:(t + 1) * PT, :], st["out"][:])
                st = nxt
                nxt = stage_a1(t + 2) if t + 2 < NT else None
    nc.compile()
    return nc


def host_prep(inputs, W):
    x = np.ascontiguousarray(inputs, np.float32).reshape(NCORES, L, NI)
    x = x.astype(np.float16)
    wsum = np.ascontiguousarray(
        W.transpose(1, 2, 0, 3).reshape(NI, KJ), np.float32)
    wsum = wsum.astype(np.float16).reshape(2, 128, KJ)
    wbd_full = np.zeros((NI, KNJ), np.float16)
    for n in range(N):
        wbd_full[n * I:(n + 1) * I, n * KJ:(n + 1) * KJ] = (
            W[:, n].transpose(1, 0, 2).reshape(I, KJ).astype(np.float16))
    wbd = np.stack([wbd_full[0:128, 0:KNJ // 2],
                    wbd_full[128:256, KNJ // 2:]])
    eye = np.eye(128, dtype=np.float16)
    return x, wsum, wbd, eye


_CACHED = {}


def kernel(inputs, W):
    from concourse.bass_utils import run_bass_kernel_spmd

    x, wsum, wbd, eye = host_prep(inputs, W)
    if "nc" not in _CACHED:
        _CACHED["nc"] = build_bass()
    nc = _CACHED["nc"]
    in_maps = [{"x": np.ascontiguousarray(x[c]), "wsum": wsum, "wbd": wbd,
                "eye": eye} for c in range(NCORES)]
    res = run_bass_kernel_spmd(nc, in_maps, core_ids=list(range(NCORES)))
    out = np.stack([res.results[c]["out"] for c in range(NCORES)])
    return out.reshape(B, R, C, K, J)
